# revision 24
# baseline (speedup 1.0000x reference)
"""Trainium2 Bass kernel for nn_GAttention (gnn_message_passing).

Reference computation (per batch b):
    q = s[:,b,:] @ Qweight                      # (N, H)
    k = Kweight.T @ s[:,b,:]                    # (H, I)   (contraction over n)
    att1 = (q @ k) * (1/sqrt(H)) + 1e-9         # (N, I)
    att2 = att1**2 @ Gmat                       # (N, I)
    out[:,b,:] = att2 / (rowsum(att2) + 1e-3)

Sharding: pure data-parallel over batch B=16 -> 2 batches per core on 8 cores.
Gmat/Qweight/Kweight replicated.

Design, driven by ntff profiles of earlier revisions:
  * DMA: every tensor is HOST-PACKED into its exact SBUF tile layout and
    moved in few large fully-contiguous dma_starts (~340GB/s vs ~170
    chunked). Inputs ride the sync HWDGE ring, outputs ride it too but
    only after all inputs are issued; their semaphore waits sit in
    otherwise-idle queues so they never block compute.
  * PE streams one moving column per cycle (~216ns per 512-wide matmul
    at 2.4GHz); fp8 DoubleRow doubles the contraction per column.
    M=64 matmuls run at half rate, so k and q are FUSED into a single
    M=128 DoubleRow chain with block-diagonal weights:
        subtile0 = [Kw | 0] paired with s_n,  subtile1 = [0 | Qw] with s_T
    -> PSUM partitions 0-63 = k, 64-127 = q, full-rate.
    s ships fp8 e4m3 with s_n/s_T chunks interleaved in one tensor.
  * att1 (K=64) would idle half the PE rows; after two SBUF->SBUF DMA
    partition remaps (building the swapped copy of k/q), att1 tiles run
    as CONCURRENT row-tiled pairs: rows 0-63 and rows 64-127.
  * Qweight/Kweight are pre-scaled by 32 on the host (fp8 normal range);
    the compensating 2^-13 = (1/8)/32/32 is applied in the att1
    eviction / square (exact powers of two). The +1e-9 is dropped
    (relative ~3e-9).
  * Vector work is spread: ACT evicts att2 po0 PSUM->f16 with fused
    accum_out rowsum + most att1 squares; DVE evicts po1 the same way +
    reciprocal + row normalization; GPSIMD adds rs0+1e-3+rs1 and issues
    nothing else heavy (its tensor ops are software-slow).

The two batches are software-pipelined: batch 1's kq/att1 phases are
woven into batch 0's att2 group stream so the PE never idles long
enough for the HAM clock gate to throttle.
"""

import sys

import numpy as np

try:  # concourse normally comes from the image's NIX_PYTHONPATH
    import concourse  # noqa: F401
except ImportError:  # pragma: no cover
    sys.path.insert(0, "/opt/trn_rl_repo")

N_DIM = 1024
IN_DIM = 1024
H_DIM = 64
B = 16
N_CORES = 8
B_LOC = B // N_CORES  # batches per core

P = 128          # SBUF/PSUM partitions
NCH = 8          # chunks over n / i (1024/128)
NH = 512         # psum free-dim half (one fp32 bank)

MODE = "B"       # fp8 everywhere the PE touches; "A" (bf16 feeds) unused

_NC_CACHE = {}


def _build_nc(mode=MODE):
    import concourse.bass as bass  # noqa: F401
    import concourse.tile as tile
    from concourse import bacc, mybir

    f32 = mybir.dt.float32
    bf16 = mybir.dt.bfloat16
    f16 = mybir.dt.float16
    f8 = mybir.dt.float8e4
    AFT = mybir.ActivationFunctionType
    DR = mybir.MatmulPerfMode.DoubleRow
    ALU = mybir.AluOpType

    assert mode == "B"
    sq_scale = 2.0 ** -13  # (1/8) / (32*32)

    nc = bacc.Bacc(
        "TRN2",
        target_bir_lowering=False,
        debug=False,
        num_devices=N_CORES,
    )
    # host-packed layouts (see _run): s has s_n / s_T chunk-interleaved
    s_d = nc.dram_tensor("s", [P, B_LOC, NCH, 2, N_DIM], f8, kind="ExternalInput")
    g_d = nc.dram_tensor("gmat", [P, NCH, IN_DIM], f8, kind="ExternalInput")
    w_d = nc.dram_tensor("wkq", [P, NCH, 2, P], f8, kind="ExternalInput")
    o_d = nc.dram_tensor("out", [P, B_LOC, NCH, IN_DIM], f16, kind="ExternalOutput")

    with tile.TileContext(nc) as tc:
        with (
            tc.tile_pool(name="const", bufs=1) as const_pool,
            tc.tile_pool(name="gmat", bufs=1) as gmat_pool,
            tc.tile_pool(name="sb", bufs=2) as s_pool,
            tc.tile_pool(name="att1e", bufs=2) as att1e_pool,
            tc.tile_pool(name="att1q", bufs=2) as att1q_pool,
            tc.tile_pool(name="kq", bufs=2) as kq_pool,
            tc.tile_pool(name="kqs", bufs=2) as kqs_pool,
            tc.tile_pool(name="outs", bufs=2) as out_pool,
            tc.tile_pool(name="stat", bufs=8) as stat_pool,
            tc.tile_pool(name="ps5", bufs=6, space="PSUM") as ps5,
            tc.tile_pool(name="psKQ", bufs=1, space="PSUM") as psKQ,
        ):
            w_sb = const_pool.tile([P, NCH, 2, P], f8)
            g_sb = gmat_pool.tile([P, NCH, IN_DIM], f8)

            def phase_load_s(b, pieces):
                """One interleaved tensor per batch; finer pieces for batch 0
                so the kq chain starts as early as possible."""
                s_t = s_pool.tile([P, NCH, 2, N_DIM], f8, tag="s")
                step = NCH // pieces
                for lo in range(0, NCH, step):
                    nc.sync.dma_start(
                        s_t[:, lo:lo + step, :, :], s_d.ap()[:, b, lo:lo + step, :, :]
                    )
                return s_t

            def kq_matmuls(s_t, ps):
                """Fused k+q chain: block-diagonal DoubleRow, M=128.
                psum rows 0-63 = k[h, :], rows 64-127 = qT[h, :].
                Ordered by column half so each half can evict early."""
                ins = []
                for half in range(2):
                    for c in range(NCH):
                        ins.append(lambda c=c, half=half: nc.tensor.matmul(
                            ps[:, half * NH:(half + 1) * NH],
                            w_sb[:, c, :, :],
                            s_t[:, c, :, half * NH:(half + 1) * NH],
                            start=(c == 0),
                            stop=(c == NCH - 1),
                            perf_mode=DR,
                        ))
                return ins

            def make_kq_tiles():
                kq_t = kq_pool.tile([P, N_DIM], bf16, tag="kq")
                kq_s = kqs_pool.tile([P, N_DIM], bf16, tag="kqs")
                return kq_t, kq_s

            def emit_evict_kq_half(ps, kq_t, kq_s, half):
                """Evict one column half of the fused kq psum, then build the
                row-swapped copy via two SBUF->SBUF partition-remap DMAs.
                kq_t rows: 0-63 k, 64-127 q;  kq_s rows: 0-63 q, 64-127 k."""
                sl = slice(half * NH, (half + 1) * NH)
                nc.vector.tensor_copy(kq_t[:, sl], ps[:, sl])
                nc.sync.dma_start(kq_s[0:H_DIM, sl], kq_t[H_DIM:P, sl])
                nc.sync.dma_start(kq_s[H_DIM:P, sl], kq_t[0:H_DIM, sl])

            att1_ctr = [0]

            def emit_att1_pair(att1e, att1sq, kq_t, kq_s, ci, half, dve_every):
                """Two att1 tiles (ci,half),(ci+1,half) as a concurrent
                row-tiled pair: rows 0-63 (k_lo x q_lo) and 64-127
                (k_hi x q_hi). Squares mostly on ACT, some on DVE."""
                pa0 = ps5.tile([P, NH], f32, tag="ps512")
                pa1 = ps5.tile([P, NH], f32, tag="ps512")
                nc.tensor.matmul(
                    pa0[:],
                    kq_t[0:H_DIM, ci * P:(ci + 1) * P],
                    kq_s[0:H_DIM, half * NH:(half + 1) * NH],
                    start=True, stop=True, tile_position=(0, 0),
                )
                nc.tensor.matmul(
                    pa1[:],
                    kq_s[H_DIM:P, (ci + 1) * P:(ci + 2) * P],
                    kq_t[H_DIM:P, half * NH:(half + 1) * NH],
                    start=True, stop=True, tile_position=(H_DIM, 0),
                )
                for cc, pa in ((ci, pa0), (ci + 1, pa1)):
                    dst = att1sq[:, cc, half * NH:(half + 1) * NH]
                    att1_ctr[0] += 1
                    if att1_ctr[0] % dve_every != 0:
                        nc.scalar.activation(dst, pa[:], AFT.Square, scale=sq_scale)
                    else:
                        stage = att1e[:, cc, half * NH:(half + 1) * NH]
                        nc.vector.tensor_scalar(
                            stage, pa[:], sq_scale, 0.0, op0=ALU.mult, op1=ALU.add,
                        )
                        nc.vector.tensor_mul(dst, stage, stage)

            def phase_att2_group(b, att1sq, out_sb, nt):
                """One att2 row-block: DoubleRow matmuls; ACT/DVE evict the
                two PSUM halves in parallel with fused rowsums; GPSIMD adds
                rs0+1e-3+rs1; DVE reciprocal + row normalization."""
                po0 = ps5.tile([P, NH], f32, tag="ps512")
                po1 = ps5.tile([P, NH], f32, tag="ps512")
                for ks in range(NCH // 2):
                    lhsT = att1sq[:, 2 * ks:2 * ks + 2, nt * P:(nt + 1) * P]
                    nc.tensor.matmul(
                        po0[:], lhsT, g_sb[:, 2 * ks:2 * ks + 2, 0:NH],
                        start=(ks == 0), stop=(ks == NCH // 2 - 1),
                        perf_mode=DR,
                    )
                    nc.tensor.matmul(
                        po1[:], lhsT, g_sb[:, 2 * ks:2 * ks + 2, NH:2 * NH],
                        start=(ks == 0), stop=(ks == NCH // 2 - 1),
                        perf_mode=DR,
                    )
                dst = out_sb[:, nt, :]
                rs0 = stat_pool.tile([P, 1], f32, tag="rs0")
                rs1 = stat_pool.tile([P, 1], f32, tag="rs1")
                nc.scalar.activation(dst[:, 0:NH], po0[:], AFT.Copy, accum_out=rs0[:])
                nc.vector.tensor_scalar(
                    dst[:, NH:2 * NH], po1[:], 1.0, 0.0,
                    op0=ALU.mult, op1=ALU.add, accum_out=rs1[:],
                )
                # rs0+rs1 on ACT (Identity with AP bias) keeps the pacing DVE
                # queue short; the reference's +1e-3 is dropped (rowsums are
                # ~5e4, relative 2e-8)
                rsum = stat_pool.tile([P, 1], f32, tag="rsum")
                nc.scalar.activation(rsum[:], rs0[:], AFT.Identity, bias=rs1[:])
                rinv = stat_pool.tile([P, 1], f32, tag="rinv")
                nc.vector.reciprocal(rinv[:], rsum[:])
                # normalize halves on different engines (keeps DVE under the
                # PE group pace)
                nc.scalar.activation(
                    dst[:, 0:NH], dst[:, 0:NH], AFT.Copy, scale=rinv[:]
                )
                nc.vector.tensor_scalar_mul(
                    dst[:, NH:2 * NH], dst[:, NH:2 * NH], rinv[:]
                )

            def emit_out_dma(b, out_sb, lo, hi):
                # sync ring is idle once inputs are issued; the trigger's
                # semaphore wait blocks nothing there
                nc.sync.dma_start(
                    o_d.ap()[:, b, lo:hi, :], out_sb[:, lo:hi, :]
                )

            ATT1_PAIRS = [(ci, half) for half in range(2) for ci in (0, 2, 4, 6)]

            # ---- DMA schedule (one FIFO ring): w -> s(b0) -> G -> s(b1)
            nc.sync.dma_start(w_sb[:], w_d.ap())
            s0 = phase_load_s(0, pieces=2)
            nc.sync.dma_start(g_sb[:], g_d.ap())
            s1 = phase_load_s(1, pieces=2)

            # ---- batch 0: fused kq (per-half evict), att1
            ps_kq0 = psKQ.tile([P, N_DIM], f32, tag="kq")
            kq0 = kq_matmuls(s0, ps_kq0)
            kq_t0, kq_s0 = make_kq_tiles()
            for m in kq0[0:NCH]:
                m()
            emit_evict_kq_half(ps_kq0, kq_t0, kq_s0, 0)
            for m in kq0[NCH:]:
                m()
            emit_evict_kq_half(ps_kq0, kq_t0, kq_s0, 1)
            att1e0 = att1e_pool.tile([P, NCH, N_DIM], bf16, tag="att1e")
            att1sq0 = att1q_pool.tile([P, NCH, N_DIM], f8, tag="att1q")
            for ci, half in ATT1_PAIRS:
                emit_att1_pair(att1e0, att1sq0, kq_t0, kq_s0, ci, half, 2)

            # ---- att2(b0) with kq/att1 of batch 1 woven into the stream
            out_sb0 = out_pool.tile([P, NCH, IN_DIM], f16, tag="out")
            ps_kq1 = psKQ.tile([P, N_DIM], f32, tag="kq")
            kq_ins = kq_matmuls(s1, ps_kq1)
            nk = len(kq_ins)
            att1e1 = att1e_pool.tile([P, NCH, N_DIM], bf16, tag="att1e")
            att1sq1 = att1q_pool.tile([P, NCH, N_DIM], f8, tag="att1q")
            holder = {}

            for nt in range(NCH):
                phase_att2_group(0, att1sq0, out_sb0, nt)
                if nt == 3:
                    emit_out_dma(0, out_sb0, 0, 4)
                elif nt == 5:
                    emit_out_dma(0, out_sb0, 4, 6)
                if nt == 1:
                    kq_t1, kq_s1 = make_kq_tiles()
                    holder["kq1"] = (kq_t1, kq_s1)
                    for m in kq_ins[0:nk // 2]:
                        m()
                    emit_evict_kq_half(ps_kq1, kq_t1, kq_s1, 0)
                elif nt == 2:
                    t, s_ = holder["kq1"]
                    for m in kq_ins[nk // 2:]:
                        m()
                    emit_evict_kq_half(ps_kq1, t, s_, 1)
                elif nt == 4:
                    t, s_ = holder["kq1"]
                    for ci, half in ATT1_PAIRS[0:3]:
                        emit_att1_pair(att1e1, att1sq1, t, s_, ci, half, 8)
                elif nt == 5:
                    t, s_ = holder["kq1"]
                    for ci, half in ATT1_PAIRS[3:5]:
                        emit_att1_pair(att1e1, att1sq1, t, s_, ci, half, 8)
                elif nt == 6:
                    t, s_ = holder["kq1"]
                    for ci, half in ATT1_PAIRS[5:7]:
                        emit_att1_pair(att1e1, att1sq1, t, s_, ci, half, 8)
                elif nt == 7:
                    t, s_ = holder["kq1"]
                    for ci, half in ATT1_PAIRS[7:8]:
                        emit_att1_pair(att1e1, att1sq1, t, s_, ci, half, 8)
            emit_out_dma(0, out_sb0, 6, 8)

            out_sb1 = out_pool.tile([P, NCH, IN_DIM], f16, tag="out")
            for nt in range(NCH):
                phase_att2_group(1, att1sq1, out_sb1, nt)
                if nt == 2:
                    emit_out_dma(1, out_sb1, 0, 3)
                elif nt == 4:
                    emit_out_dma(1, out_sb1, 3, 5)
                elif nt == 5:
                    emit_out_dma(1, out_sb1, 5, 6)
                elif nt == 6:
                    emit_out_dma(1, out_sb1, 6, 7)
            emit_out_dma(1, out_sb1, 7, 8)

    nc.compile()
    return nc


def _get_nc(mode=MODE):
    if mode not in _NC_CACHE:
        _NC_CACHE[mode] = _build_nc(mode)
    return _NC_CACHE[mode]


def _pack(a):
    """[C*P, ...tail] -> [P, C, ...tail]: row c*P+p -> [p][c]."""
    c = a.shape[0] // P
    return np.ascontiguousarray(
        a.reshape(c, P, *a.shape[1:]).swapaxes(0, 1)
    )


def _run(inputs, trace=False, mm_mode=None, tmpdir=None, mode=MODE):
    import ml_dtypes
    from concourse.bass_utils import run_bass_kernel_spmd

    f8 = ml_dtypes.float8_e4m3

    s = np.asarray(inputs["s"], dtype=np.float32)
    g8 = _pack(np.asarray(inputs["Gmat"], np.float32).astype(f8))
    qw = (np.asarray(inputs["Qweight"], np.float32) * 32.0).astype(f8)
    kw = (np.asarray(inputs["Kweight"], np.float32) * 32.0).astype(f8)

    # block-diagonal fused kq weights: [P, NCH, 2, P]
    #   subtile0 (pairs s_n): cols 0-63 = Kw, cols 64-127 = 0
    #   subtile1 (pairs s_T): cols 0-63 = 0,  cols 64-127 = Qw
    wkq = np.zeros((P, NCH, 2, P), f8)
    kw_p = _pack(kw)   # [P, NCH, H]
    qw_p = _pack(qw)
    wkq[:, :, 0, 0:H_DIM] = kw_p
    wkq[:, :, 1, H_DIM:P] = qw_p

    s_c = s.astype(f8)

    nc = _get_nc(mode)
    in_maps = []
    for c in range(N_CORES):
        sl = s_c[:, c * B_LOC:(c + 1) * B_LOC, :]            # [N, B_LOC, I]
        sn = _pack(sl)                                        # [P, NCH, B_LOC, I]
        st = _pack(np.ascontiguousarray(sl.transpose(2, 1, 0)))
        # s[p, b, ch, 0, :] = s_n chunk, s[p, b, ch, 1, :] = s_T chunk
        sb = np.empty((P, B_LOC, NCH, 2, N_DIM), f8)
        sb[:, :, :, 0, :] = sn.transpose(0, 2, 1, 3)
        sb[:, :, :, 1, :] = st.transpose(0, 2, 1, 3)
        in_maps.append({
            "s": sb,
            "gmat": g8,
            "wkq": wkq,
        })
    res = run_bass_kernel_spmd(
        nc, in_maps, list(range(N_CORES)), trace=trace, tmpdir=tmpdir
    )
    outs = []
    for c in range(N_CORES):
        o = res.results[c]["out"]                             # [P, B_LOC, NCH, I]
        outs.append(o.transpose(2, 0, 1, 3).reshape(N_DIM, B_LOC, IN_DIM))
    out = np.concatenate(outs, axis=1).astype(np.float32)
    return out, res


def kernel(**inputs) -> np.ndarray:
    out, _ = _run(inputs, trace=False)
    return out


# revision 28
# speedup vs baseline: 1.1045x; 1.1045x over previous
"""Trainium2 Bass kernel for nn_GAttention (gnn_message_passing).

Reference computation (per batch b):
    q = s[:,b,:] @ Qweight                      # (N, H)
    k = Kweight.T @ s[:,b,:]                    # (H, I)   (contraction over n)
    att1 = (q @ k) * (1/sqrt(H)) + 1e-9         # (N, I)
    att2 = att1**2 @ Gmat                       # (N, I)
    out[:,b,:] = att2 / (rowsum(att2) + 1e-3)

Sharding: pure data-parallel over batch B=16 -> 2 batches per core on 8 cores.
Gmat/Qweight/Kweight replicated.

Design, driven by ntff profiles of earlier revisions:
  * DMA: every tensor is HOST-PACKED into its exact SBUF tile layout and
    moved in few large fully-contiguous dma_starts (~340GB/s vs ~170
    chunked). Inputs ride the sync HWDGE ring, outputs ride it too but
    only after all inputs are issued; their semaphore waits sit in
    otherwise-idle queues so they never block compute.
  * PE streams one moving column per cycle (~216ns per 512-wide matmul
    at 2.4GHz); fp8 DoubleRow doubles the contraction per column.
    M=64 matmuls run at half rate, so k and q are FUSED into a single
    M=128 DoubleRow chain with block-diagonal weights:
        subtile0 = [Kw | 0] paired with s_n,  subtile1 = [0 | Qw] with s_T
    -> PSUM partitions 0-63 = k, 64-127 = q, full-rate.
    s ships fp8 e4m3 with s_n/s_T chunks interleaved in one tensor.
  * att1 (K=64) would idle half the PE rows; after two SBUF->SBUF DMA
    partition remaps (building the swapped copy of k/q), att1 tiles run
    as CONCURRENT row-tiled pairs: rows 0-63 and rows 64-127.
  * Qweight/Kweight are pre-scaled by 32 on the host (fp8 normal range);
    the compensating 2^-13 = (1/8)/32/32 is applied in the att1
    eviction / square (exact powers of two). The +1e-9 is dropped
    (relative ~3e-9).
  * Vector work is spread: ACT evicts att2 po0 PSUM->f16 with fused
    accum_out rowsum + most att1 squares; DVE evicts po1 the same way +
    reciprocal + row normalization; GPSIMD adds rs0+1e-3+rs1 and issues
    nothing else heavy (its tensor ops are software-slow).

The two batches are software-pipelined: batch 1's kq/att1 phases are
woven into batch 0's att2 group stream so the PE never idles long
enough for the HAM clock gate to throttle.
"""

import sys

import numpy as np

try:  # concourse normally comes from the image's NIX_PYTHONPATH
    import concourse  # noqa: F401
except ImportError:  # pragma: no cover
    sys.path.insert(0, "/opt/trn_rl_repo")

N_DIM = 1024
IN_DIM = 1024
H_DIM = 64
B = 16
N_CORES = 8
B_LOC = B // N_CORES  # batches per core

P = 128          # SBUF/PSUM partitions
NCH = 8          # chunks over n / i (1024/128)
NH = 512         # psum free-dim half (one fp32 bank)

MODE = "B"       # fp8 everywhere the PE touches; "A" (bf16 feeds) unused

_NC_CACHE = {}


def _build_nc(mode=MODE):
    import concourse.bass as bass  # noqa: F401
    import concourse.tile as tile
    from concourse import bacc, mybir

    f32 = mybir.dt.float32
    bf16 = mybir.dt.bfloat16
    f16 = mybir.dt.float16
    f8 = mybir.dt.float8e4
    AFT = mybir.ActivationFunctionType
    DR = mybir.MatmulPerfMode.DoubleRow
    ALU = mybir.AluOpType

    assert mode == "B"
    sq_scale = 2.0 ** -13  # (1/8) / (32*32)

    nc = bacc.Bacc(
        "TRN2",
        target_bir_lowering=False,
        debug=False,
        num_devices=N_CORES,
    )
    # host-packed layouts (see _run): s has s_n / s_T chunk-interleaved
    s_d = nc.dram_tensor("s", [P, B_LOC, NCH, 2, N_DIM], f8, kind="ExternalInput")
    g_d = nc.dram_tensor("gmat", [P, NCH, IN_DIM], f8, kind="ExternalInput")
    w_d = nc.dram_tensor("wkq", [P, NCH, 2, P], f8, kind="ExternalInput")
    o_d = nc.dram_tensor("out", [P, B_LOC, NCH, IN_DIM], f16, kind="ExternalOutput")

    with tile.TileContext(nc) as tc:
        with (
            tc.tile_pool(name="const", bufs=1) as const_pool,
            tc.tile_pool(name="gmat", bufs=1) as gmat_pool,
            tc.tile_pool(name="sb", bufs=2) as s_pool,
            tc.tile_pool(name="att1e", bufs=2) as att1e_pool,
            tc.tile_pool(name="att1q", bufs=2) as att1q_pool,
            tc.tile_pool(name="kq", bufs=2) as kq_pool,
            tc.tile_pool(name="kqs", bufs=2) as kqs_pool,
            tc.tile_pool(name="outs", bufs=2) as out_pool,
            tc.tile_pool(name="stat", bufs=8) as stat_pool,
            tc.tile_pool(name="ps5", bufs=6, space="PSUM") as ps5,
            tc.tile_pool(name="psKQ", bufs=1, space="PSUM") as psKQ,
        ):
            w_sb = const_pool.tile([P, NCH, 2, P], f8)
            g_sb = gmat_pool.tile([P, NCH, IN_DIM], f8)

            def phase_load_s(b, pieces):
                """One interleaved tensor per batch; finer pieces for batch 0
                so the kq chain starts as early as possible."""
                s_t = s_pool.tile([P, NCH, 2, N_DIM], f8, tag="s")
                step = NCH // pieces
                for lo in range(0, NCH, step):
                    nc.sync.dma_start(
                        s_t[:, lo:lo + step, :, :], s_d.ap()[:, b, lo:lo + step, :, :]
                    )
                return s_t

            def kq_matmuls(s_t, ps):
                """Fused k+q chain: block-diagonal DoubleRow, M=128.
                psum rows 0-63 = k[h, :], rows 64-127 = qT[h, :].
                Ordered by column half so each half can evict early."""
                ins = []
                for half in range(2):
                    for c in range(NCH):
                        ins.append(lambda c=c, half=half: nc.tensor.matmul(
                            ps[:, half * NH:(half + 1) * NH],
                            w_sb[:, c, :, :],
                            s_t[:, c, :, half * NH:(half + 1) * NH],
                            start=(c == 0),
                            stop=(c == NCH - 1),
                            perf_mode=DR,
                        ))
                return ins

            def make_kq_tiles():
                kq_t = kq_pool.tile([P, N_DIM], bf16, tag="kq")
                kq_s = kqs_pool.tile([P, N_DIM], bf16, tag="kqs")
                return kq_t, kq_s

            def emit_evict_kq_half(ps, kq_t, kq_s, half):
                """Evict one column half of the fused kq psum, then build the
                row-swapped copy via two SBUF->SBUF partition-remap DMAs.
                kq_t rows: 0-63 k, 64-127 q;  kq_s rows: 0-63 q, 64-127 k."""
                sl = slice(half * NH, (half + 1) * NH)
                nc.vector.tensor_copy(kq_t[:, sl], ps[:, sl])
                nc.sync.dma_start(kq_s[0:H_DIM, sl], kq_t[H_DIM:P, sl])
                nc.sync.dma_start(kq_s[H_DIM:P, sl], kq_t[0:H_DIM, sl])

            att1_ctr = [0]

            def emit_att1_pair(att1e, att1sq, kq_t, kq_s, ci, half, dve_every):
                """Two att1 tiles (ci,half),(ci+1,half) as a concurrent
                row-tiled pair: rows 0-63 (k_lo x q_lo) and 64-127
                (k_hi x q_hi). Squares mostly on ACT, some on DVE."""
                pa0 = ps5.tile([P, NH], f32, tag="ps512")
                pa1 = ps5.tile([P, NH], f32, tag="ps512")
                nc.tensor.matmul(
                    pa0[:],
                    kq_t[0:H_DIM, ci * P:(ci + 1) * P],
                    kq_s[0:H_DIM, half * NH:(half + 1) * NH],
                    start=True, stop=True, tile_position=(0, 0),
                )
                nc.tensor.matmul(
                    pa1[:],
                    kq_s[H_DIM:P, (ci + 1) * P:(ci + 2) * P],
                    kq_t[H_DIM:P, half * NH:(half + 1) * NH],
                    start=True, stop=True, tile_position=(H_DIM, 0),
                )
                for cc, pa in ((ci, pa0), (ci + 1, pa1)):
                    dst = att1sq[:, cc, half * NH:(half + 1) * NH]
                    att1_ctr[0] += 1
                    if att1_ctr[0] % dve_every != 0:
                        nc.scalar.activation(dst, pa[:], AFT.Square, scale=sq_scale)
                    else:
                        stage = att1e[:, cc, half * NH:(half + 1) * NH]
                        nc.vector.tensor_scalar(
                            stage, pa[:], sq_scale, 0.0, op0=ALU.mult, op1=ALU.add,
                        )
                        nc.vector.tensor_mul(dst, stage, stage)

            def phase_att2_group(b, att1sq, out_sb, nt, last=False):
                """One att2 row-block: DoubleRow matmuls; ACT/DVE evict the
                two PSUM halves in parallel with fused rowsums; ACT adds the
                rowsums; DVE reciprocal + row normalization. The last group
                of the kernel evicts both halves on ACT (shorter DVE tail,
                and there is nothing behind it to block)."""
                po0 = ps5.tile([P, NH], f32, tag="ps512")
                po1 = ps5.tile([P, NH], f32, tag="ps512")
                for ks in range(NCH // 2):
                    lhsT = att1sq[:, 2 * ks:2 * ks + 2, nt * P:(nt + 1) * P]
                    nc.tensor.matmul(
                        po0[:], lhsT, g_sb[:, 2 * ks:2 * ks + 2, 0:NH],
                        start=(ks == 0), stop=(ks == NCH // 2 - 1),
                        perf_mode=DR,
                    )
                    nc.tensor.matmul(
                        po1[:], lhsT, g_sb[:, 2 * ks:2 * ks + 2, NH:2 * NH],
                        start=(ks == 0), stop=(ks == NCH // 2 - 1),
                        perf_mode=DR,
                    )
                dst = out_sb[:, nt, :]
                rs0 = stat_pool.tile([P, 1], f32, tag="rs0")
                rs1 = stat_pool.tile([P, 1], f32, tag="rs1")
                nc.scalar.activation(dst[:, 0:NH], po0[:], AFT.Copy, accum_out=rs0[:])
                if last:
                    nc.scalar.activation(
                        dst[:, NH:2 * NH], po1[:], AFT.Copy, accum_out=rs1[:]
                    )
                else:
                    nc.vector.tensor_scalar(
                        dst[:, NH:2 * NH], po1[:], 1.0, 0.0,
                        op0=ALU.mult, op1=ALU.add, accum_out=rs1[:],
                    )
                # rs0+rs1 on ACT (Identity with AP bias) keeps the pacing DVE
                # queue short; the reference's +1e-3 is dropped (rowsums are
                # ~5e4, relative 2e-8)
                rsum = stat_pool.tile([P, 1], f32, tag="rsum")
                nc.scalar.activation(rsum[:], rs0[:], AFT.Identity, bias=rs1[:])
                rinv = stat_pool.tile([P, 1], f32, tag="rinv")
                nc.vector.reciprocal(rinv[:], rsum[:])
                nc.vector.tensor_scalar_mul(dst, dst, rinv[:])

            def emit_out_dma(b, out_sb, lo, hi):
                # sync ring is idle once inputs are issued; the trigger's
                # semaphore wait blocks nothing there
                nc.sync.dma_start(
                    o_d.ap()[:, b, lo:hi, :], out_sb[:, lo:hi, :]
                )

            ATT1_PAIRS = [(ci, half) for half in range(2) for ci in (0, 2, 4, 6)]

            # ---- DMA schedule (one FIFO ring): w -> s(b0) -> G -> s(b1)
            nc.sync.dma_start(w_sb[:], w_d.ap())
            s0 = phase_load_s(0, pieces=2)
            nc.sync.dma_start(g_sb[:], g_d.ap())
            s1 = phase_load_s(1, pieces=2)

            # ---- batch 0: fused kq (per-half evict), att1
            ps_kq0 = psKQ.tile([P, N_DIM], f32, tag="kq")
            kq0 = kq_matmuls(s0, ps_kq0)
            kq_t0, kq_s0 = make_kq_tiles()
            for m in kq0[0:NCH]:
                m()
            emit_evict_kq_half(ps_kq0, kq_t0, kq_s0, 0)
            for m in kq0[NCH:]:
                m()
            emit_evict_kq_half(ps_kq0, kq_t0, kq_s0, 1)
            att1e0 = att1e_pool.tile([P, NCH, N_DIM], bf16, tag="att1e")
            att1sq0 = att1q_pool.tile([P, NCH, N_DIM], f8, tag="att1q")
            for ci, half in ATT1_PAIRS:
                emit_att1_pair(att1e0, att1sq0, kq_t0, kq_s0, ci, half, 3)

            # ---- att2(b0) with kq/att1 of batch 1 woven into the stream
            out_sb0 = out_pool.tile([P, NCH, IN_DIM], f16, tag="out")
            ps_kq1 = psKQ.tile([P, N_DIM], f32, tag="kq")
            kq_ins = kq_matmuls(s1, ps_kq1)
            nk = len(kq_ins)
            att1e1 = att1e_pool.tile([P, NCH, N_DIM], bf16, tag="att1e")
            att1sq1 = att1q_pool.tile([P, NCH, N_DIM], f8, tag="att1q")
            holder = {}

            for nt in range(NCH):
                phase_att2_group(0, att1sq0, out_sb0, nt)
                if nt == 3:
                    emit_out_dma(0, out_sb0, 0, 4)
                elif nt == 5:
                    emit_out_dma(0, out_sb0, 4, 6)
                if nt == 1:
                    kq_t1, kq_s1 = make_kq_tiles()
                    holder["kq1"] = (kq_t1, kq_s1)
                    for m in kq_ins[0:nk // 2]:
                        m()
                    emit_evict_kq_half(ps_kq1, kq_t1, kq_s1, 0)
                elif nt == 2:
                    t, s_ = holder["kq1"]
                    for m in kq_ins[nk // 2:]:
                        m()
                    emit_evict_kq_half(ps_kq1, t, s_, 1)
                elif nt == 4:
                    t, s_ = holder["kq1"]
                    for ci, half in ATT1_PAIRS[0:3]:
                        emit_att1_pair(att1e1, att1sq1, t, s_, ci, half, 8)
                elif nt == 5:
                    t, s_ = holder["kq1"]
                    for ci, half in ATT1_PAIRS[3:5]:
                        emit_att1_pair(att1e1, att1sq1, t, s_, ci, half, 8)
                elif nt == 6:
                    t, s_ = holder["kq1"]
                    for ci, half in ATT1_PAIRS[5:7]:
                        emit_att1_pair(att1e1, att1sq1, t, s_, ci, half, 8)
                elif nt == 7:
                    t, s_ = holder["kq1"]
                    for ci, half in ATT1_PAIRS[7:8]:
                        emit_att1_pair(att1e1, att1sq1, t, s_, ci, half, 8)
            emit_out_dma(0, out_sb0, 6, 8)

            out_sb1 = out_pool.tile([P, NCH, IN_DIM], f16, tag="out")
            for nt in range(NCH):
                phase_att2_group(1, att1sq1, out_sb1, nt, last=(nt == NCH - 1))
                if nt == 2:
                    emit_out_dma(1, out_sb1, 0, 3)
                elif nt == 4:
                    emit_out_dma(1, out_sb1, 3, 5)
                elif nt == 5:
                    emit_out_dma(1, out_sb1, 5, 6)
                elif nt == 6:
                    emit_out_dma(1, out_sb1, 6, 7)
            emit_out_dma(1, out_sb1, 7, 8)

    nc.compile()
    return nc


def _get_nc(mode=MODE):
    if mode not in _NC_CACHE:
        _NC_CACHE[mode] = _build_nc(mode)
    return _NC_CACHE[mode]


def _pack(a):
    """[C*P, ...tail] -> [P, C, ...tail]: row c*P+p -> [p][c]."""
    c = a.shape[0] // P
    return np.ascontiguousarray(
        a.reshape(c, P, *a.shape[1:]).swapaxes(0, 1)
    )


def _run(inputs, trace=False, mm_mode=None, tmpdir=None, mode=MODE):
    import ml_dtypes
    from concourse.bass_utils import run_bass_kernel_spmd

    f8 = ml_dtypes.float8_e4m3

    s = np.asarray(inputs["s"], dtype=np.float32)
    g8 = _pack(np.asarray(inputs["Gmat"], np.float32).astype(f8))
    qw = (np.asarray(inputs["Qweight"], np.float32) * 32.0).astype(f8)
    kw = (np.asarray(inputs["Kweight"], np.float32) * 32.0).astype(f8)

    # block-diagonal fused kq weights: [P, NCH, 2, P]
    #   subtile0 (pairs s_n): cols 0-63 = Kw, cols 64-127 = 0
    #   subtile1 (pairs s_T): cols 0-63 = 0,  cols 64-127 = Qw
    wkq = np.zeros((P, NCH, 2, P), f8)
    kw_p = _pack(kw)   # [P, NCH, H]
    qw_p = _pack(qw)
    wkq[:, :, 0, 0:H_DIM] = kw_p
    wkq[:, :, 1, H_DIM:P] = qw_p

    s_c = s.astype(f8)

    nc = _get_nc(mode)
    in_maps = []
    for c in range(N_CORES):
        sl = s_c[:, c * B_LOC:(c + 1) * B_LOC, :]            # [N, B_LOC, I]
        sn = _pack(sl)                                        # [P, NCH, B_LOC, I]
        st = _pack(np.ascontiguousarray(sl.transpose(2, 1, 0)))
        # s[p, b, ch, 0, :] = s_n chunk, s[p, b, ch, 1, :] = s_T chunk
        sb = np.empty((P, B_LOC, NCH, 2, N_DIM), f8)
        sb[:, :, :, 0, :] = sn.transpose(0, 2, 1, 3)
        sb[:, :, :, 1, :] = st.transpose(0, 2, 1, 3)
        in_maps.append({
            "s": sb,
            "gmat": g8,
            "wkq": wkq,
        })
    res = run_bass_kernel_spmd(
        nc, in_maps, list(range(N_CORES)), trace=trace, tmpdir=tmpdir
    )
    outs = []
    for c in range(N_CORES):
        o = res.results[c]["out"]                             # [P, B_LOC, NCH, I]
        outs.append(o.transpose(2, 0, 1, 3).reshape(N_DIM, B_LOC, IN_DIM))
    out = np.concatenate(outs, axis=1).astype(np.float32)
    return out, res


def kernel(**inputs) -> np.ndarray:
    out, _ = _run(inputs, trace=False)
    return out
